# revision 27
# baseline (speedup 1.0000x reference)
"""Trainium2 Bass kernel for the Attention3 module (B=128, S=1024, RNN=2048, HID=512).

Strategy: data-parallel over batch B across 8 NeuronCores (16 batches/core),
plus structural optimizations:

  * Mask compaction (sparse attention): positions with mask==1 get softmax
    weight exactly 0, so their att_feats / p_att_feats rows are never read.
    The host gathers the unmasked rows per batch into a virtual score axis of
    SE=640 columns.  Each batch is padded to 576 positions; the tail 64 of a
    batch PAIR share one feature tile: the even batch's tail sits at score
    columns 512-575 (tile-4 partitions 0-63), the odd batch's at columns
    576-639 (partitions 64-127), so one [128, RNN] att_feats tile and one
    matmul serve both batches' tails.  Pad positions get an additive -1e9
    score so exp() zeroes them.
  * fp8 (e3m4) storage for att_feats, p_att_feats^T and the folded MLP
    weight.  The PE accepts mixed bf16-stationary x fp8-moving matmuls, and
    ScalarE auto-upconverts the fp8 tanh input.  Rel err ~1.4e-2 (gate 2e-2).
  * The 4 MLP layers have no nonlinearity between them, so the host folds
    W4@W3@W2@W1 into one [512, 2048] matrix, scaled by 256 to clear the fp8
    subnormal range (un-scaled during PSUM evacuation).
  * Softmax skips the max-subtraction: real scores are O(1) (sum of 512
    tanh-bounded terms times 0.02-scale weights), so exp() cannot overflow,
    and the -1e9 padding still underflows to exactly 0.  This shortens the
    serial softmax chain at each group boundary.

All large streams are stored PRE-TILED in HBM (partition-major within each
DMA tile) so every DMA descriptor is a 0.5-4 KiB contiguous per-partition
line.

Per-core device pipeline: batches run in groups of sizes [2,2,4,4,2,2]
(small groups at the ends shorten the fill and drain of the two-stage
scores->weighted-sum pipeline; big groups in the middle keep PE streaks
long).  Scores of group g overlap the weighted sum of group g-1; each
weighted-sum batch is emitted in two halves around the next scores batch so
the PE always has independent work queued across a group's softmax chain.
  1. MLP: att_h = h@Wc.T + bc (PE, f32 acc), bias folded in as a K=1
     ones-outer-product matmul appended to the same PSUM group.
  2. scores: tanh(p_att^T + att_h) with HID on partitions; ScalarE reads the
     fp8 p tile and writes a bf16 tile with att_h as per-partition bias; the
     Wa contraction is a PE matmul whose stationary column holds Wa masked to
     batch b, so each batch accumulates into its own PSUM row.
  3. softmax: exp (unnormalized, no max-sub) straight out of PSUM; the exp
     output is PE-transposed onto the block-diagonal of the masked weight
     tensor (full tiles) and onto per-pair split columns (shared tail tile);
     1/sum is folded into the final PSUM evacuation.
  4. weighted sum: stream compacted fp8 att_feats tiles and matmul against
     the bf16 block-diagonal weights; each batch lands in its own PSUM row.

DMA queues: wct/bc then pt tiles + consts ride the ACT HWDGE ring
(nc.scalar), so the MLP inputs land inside ~10us; f tiles round-robin
sync/gpsimd.
"""

import functools

import ml_dtypes
import numpy as np

import concourse.bacc as bacc
import concourse.bass as bass
import concourse.tile as tile
from concourse import mybir
from concourse.bass_utils import run_bass_kernel_spmd
from concourse.masks import make_identity

N_CORES = 8
B, S, RNN, HID = 128, 1024, 2048, 512
BPC = B // N_CORES  # batches per core
NPAIR = BPC // 2  # tail-sharing batch pairs
GROUPS = ((0, 2), (2, 2), (4, 4), (8, 4), (12, 2), (14, 2))  # (start, size)
NGRP = len(GROUPS)
GSMAX = 4
F32 = mybir.dt.float32
BF16 = mybir.dt.bfloat16
FP8 = mybir.dt.float8e3
FP8E4 = mybir.dt.float8e4
NP_FP8 = ml_dtypes.float8_e3m4
NP_FP8E4 = ml_dtypes.float8_e4m3
MASK_NEG = -1.0e9
WC_SCALE = 256.0
SE = 640  # virtual score-axis length
SB = 576  # per-batch padded position count
NFULL = 4  # full 128-position feature tiles per batch
AX_X = mybir.AxisListType.X
TANH = mybir.ActivationFunctionType.Tanh
EXP = mybir.ActivationFunctionType.Exp
COPY = mybir.ActivationFunctionType.Copy

NHT = HID // 128  # 4 h-tiles
NN = RNN // 512  # 4 output chunks
KT = RNN // 128  # 16 k-tiles in the MLP contraction
NWC = 4  # wct DMA chunks (4 k-tiles each)
FTS_PRIV = ((0, 2), (2, 2))  # private f tiles per batch: (start, count)

PRE_PT = 3  # p tiles issued this many batches ahead
PRE_FT = 3  # f tiles issued this many batches ahead

# batch -> (group_index, start, size, offset_in_group)
_B2G = {}
for _gi, (_b0, _gs) in enumerate(GROUPS):
    for _bl in range(_gs):
        _B2G[_b0 + _bl] = (_gi, _b0, _gs, _bl)


def _build_body(ctx, tc, io):
    nc = tc.nc

    consts = ctx.enter_context(tc.tile_pool(name="consts", bufs=1))
    wpool = ctx.enter_context(tc.tile_pool(name="wpool", bufs=4))
    mlp = ctx.enter_context(tc.tile_pool(name="mlp", bufs=1))
    ppool = ctx.enter_context(tc.tile_pool(name="ppool", bufs=5))
    # ft DMAs ride only sync/gpsimd (never a compute-critical queue: a
    # ring-full ft dma_start waiting on its matvec consumer must not block
    # ops that the matvec itself depends on).
    fpool = ctx.enter_context(tc.tile_pool(name="fpool", bufs=16))
    psA = ctx.enter_context(tc.tile_pool(name="psA", bufs=4, space="PSUM"))
    psB = ctx.enter_context(tc.tile_pool(name="psB", bufs=4, space="PSUM"))

    # ---- the MLP-critical inputs go out first, on the fast ACT ring ----
    wct_sb = []
    for wc in range(NWC):
        wt = wpool.tile([128, 2, 2, HID], FP8E4, tag="wt", name=f"wt{wc}")
        nc.scalar.dma_start(out=wt, in_=io["wct"][wc])
        wct_sb.append(wt)
    bc_sb = consts.tile([1, HID], BF16)
    nc.scalar.dma_start(out=bc_sb, in_=io["bc"])
    hT_sb = consts.tile([128, KT // 2, 2, BPC], FP8E4)
    nc.sync.dma_start(out=hT_sb, in_=io["hT"])

    # ---- constants / small inputs ----
    ident = consts.tile([128, 128], F32)
    make_identity(nc, ident)
    ones_f = consts.tile([1, BPC], F32)
    nc.vector.memset(ones_f, 1.0)
    ones1 = consts.tile([1, BPC], BF16)
    nc.vector.tensor_copy(out=ones1, in_=ones_f)

    # ---- phase 1: folded MLP, one layer (DoubleRow fp8, f32 accumulate;
    # this runs cold at mid p-state so halving the rows matters) ----
    ps_ah = psA.tile([BPC, HID], F32, tag="ps_small", name="ps_ah")
    for wc in range(NWC):
        for kpi in range(2):
            kp = wc * 2 + kpi
            nc.tensor.matmul(
                ps_ah,
                lhsT=hT_sb[:, kp, :, :],
                rhs=wct_sb[wc][:, kpi, :, :],
                start=(kp == 0),
                stop=False,
                perf_mode=mybir.MatmulPerfMode.DoubleRow,
            )
    # bias last so bc's DMA stays off the critical path; wct is scaled by
    # WC_SCALE (fp8 subnormal avoidance), bc rides in pre-scaled, and the
    # evacuation divides both out.
    nc.tensor.matmul(ps_ah, lhsT=ones1, rhs=bc_sb, start=False, stop=True)
    ah = mlp.tile([BPC, HID], F32, tag="ah")
    nc.vector.tensor_scalar_mul(out=ah, in0=ps_ah, scalar1=1.0 / WC_SCALE)
    ahT = mlp.tile([128, NHT, BPC], F32, tag="ahT")
    for j in range(NHT):
        ps = psA.tile([128, BPC], F32, tag="ps_small", name=f"ps_tr_ah{j}")
        nc.tensor.transpose(ps, ah[:, j * 128 : (j + 1) * 128], ident[:BPC, :BPC])
        nc.vector.tensor_copy(out=ahT[:, j, :], in_=ps)

    # Block-diagonal masked softmax weights, zeroed early: w_mask[:, t, b, m]
    # = exp_w[s, b] if m == b else 0 (full tiles t<4); w_sh[:, pi, m] holds
    # the pair-shared tail tile (rows 0-63 even batch, 64-127 odd batch).
    w_mask = mlp.tile([128, NFULL, BPC, BPC], BF16, tag="w_mask")
    nc.vector.memset(w_mask, 0.0)
    w_sh = mlp.tile([128, NPAIR, GSMAX], BF16, tag="w_sh")
    nc.vector.memset(w_sh, 0.0)

    wa_sb = consts.tile([128, NHT * BPC * BPC], BF16)
    nc.sync.dma_start(out=wa_sb, in_=io["warep"])
    wa_m = wa_sb.rearrange("p (t b m) -> p t b m", t=NHT, b=BPC)

    madd_sb = consts.tile([GSMAX, NGRP, SE], BF16)
    nc.sync.dma_start(out=madd_sb, in_=io["madd"])

    # Per-group state for the batch-interleaved pipeline below.
    sc_state = {}
    mv_state = {}
    rs_g = {}
    pt_tiles = {}
    ptt_tiles = {}
    ft_tiles = {}
    fsh_tiles = {}
    ft_ctr = [0]
    f_engs = [nc.sync, nc.gpsimd]

    def fq():
        eng = f_engs[ft_ctr[0] % 2]
        ft_ctr[0] += 1
        return eng

    def emit_pt_dma(b):
        # pt rides the ACT ring: it must never queue behind the ft stream,
        # whose ring-full waits would delay the tanh chain
        pt = ppool.tile([128, NHT, SE], FP8, tag="pt", name=f"pt_{b}")
        nc.scalar.dma_start(out=pt, in_=io["pT"][b])
        pt_tiles[b] = pt

    def emit_ft_dma(b):
        """Private f tiles for batch b + the shared tail tile on odd b."""
        for ti, (s0, fu) in enumerate(FTS_PRIV):
            ft = fpool.tile([128, 2, RNN], FP8, tag="ft", name=f"ft_{b}_{ti}")
            fq().dma_start(out=ft, in_=io["f2"][b, ti])
            ft_tiles[(b, ti)] = ft
        if b % 2 == 1:
            pi = b // 2
            fsh = fpool.tile([128, RNN], FP8, tag="fsh", bufs=4, name=f"fsh_{pi}")
            fq().dma_start(out=fsh, in_=io["fsh"][pi])
            fsh_tiles[pi] = fsh

    def emit_tanh(b):
        # Even batches only occupy score columns [0, 576); odd ones also use
        # the [576, 640) tail (their interior [512, 576) stays pad-garbage,
        # zeroed later by exp(-1e9)).
        pt = pt_tiles.pop(b)
        ptt = ppool.tile([128, NHT, SE], BF16, tag="ptt", bufs=4, name=f"ptt_{b}")
        hi = SB if b % 2 == 0 else SE
        for ht in range(NHT):
            nc.scalar.activation(
                out=ptt[:, ht, 0:hi],
                in_=pt[:, ht, 0:hi],
                func=TANH,
                bias=ahT[:, ht, b : b + 1],
                scale=1.0,
            )
        ptt_tiles[b] = ptt

    def emit_scores_batch(b):
        gi, b0, gs, bl = _B2G[b]
        if gi not in sc_state:
            sc_state[gi] = {
                "m": psA.tile([GSMAX, 512], F32, tag="ps_small", name=f"ps_sc_{gi}"),
                "e": psA.tile([GSMAX, 64], F32, tag="ps_small", name=f"ps_se_{gi}"),
                "o": psA.tile([GSMAX, 64], F32, tag="ps_small", name=f"ps_so_{gi}"),
            }
        ps = sc_state[gi]
        ptt = ptt_tiles.pop(b)
        even = b % 2 == 0
        off, pst = (512, ps["e"]) if even else (SB, ps["o"])
        for ht in range(NHT):
            nc.tensor.matmul(
                ps["m"][:gs],
                lhsT=wa_m[:, ht, b, b0 : b0 + gs],
                rhs=ptt[:, ht, 0:512],
                start=(bl == 0 and ht == 0),
                stop=(bl == gs - 1 and ht == NHT - 1),
            )
            nc.tensor.matmul(
                pst[:gs],
                lhsT=wa_m[:, ht, b, b0 : b0 + gs],
                rhs=ptt[:, ht, off : off + 64],
                start=(bl == (0 if even else 1) and ht == 0),
                stop=(bl == (gs - 2 if even else gs - 1) and ht == NHT - 1),
            )

    def finish_scores(gi):
        """Evacuate score PSUM (+mask), exp without max-sub, write the
        masked-weight diagonals."""
        b0, gs = GROUPS[gi]
        ps = sc_state[gi]
        scores = mlp.tile([GSMAX, SE], F32, tag="scores", bufs=2, name=f"scores{gi}")
        for off, w, key in ((0, 512, "m"), (512, 64, "e"), (SB, 64, "o")):
            nc.vector.tensor_add(
                out=scores[:gs, off : off + w],
                in0=ps[key][:gs],
                in1=madd_sb[:gs, gi, off : off + w],
            )
        ssum = mlp.tile([GSMAX, 1], F32, tag="ssum", bufs=2, name=f"ssum{gi}")
        nc.scalar.activation(
            out=scores[:gs], in_=scores[:gs], func=EXP, scale=1.0, accum_out=ssum[:gs]
        )
        rs = mlp.tile([GSMAX, 1], F32, tag="rs", bufs=2, name=f"rs{gi}")
        nc.vector.reciprocal(out=rs[:gs], in_=ssum[:gs])
        rs_g[gi] = rs
        for t in range(NFULL):
            ps_t = psA.tile([128, GSMAX], F32, tag="ps_small", name=f"ps_tr{gi}_{t}")
            nc.tensor.transpose(
                ps_t[:, :gs], scores[:gs, t * 128 : (t + 1) * 128], ident[:gs, :gs]
            )
            sl = w_mask[:, t, :, :]
            diag_ap = bass.AP(
                tensor=sl.tensor,
                offset=sl.offset + b0 * (BPC + 1),
                ap=[sl.ap[0], [BPC + 1, gs]],
            )
            nc.vector.tensor_copy(out=diag_ap, in_=ps_t[:, :gs])
        # shared tail tile: columns 512-639 transpose to partitions 0-127;
        # rows 0-63 belong to the even batch of each pair, 64-127 to the odd.
        ps_t = psA.tile([128, GSMAX], F32, tag="ps_small", name=f"ps_trs{gi}")
        nc.tensor.transpose(ps_t[:, :gs], scores[:gs, 512:SE], ident[:gs, :gs])
        for half in range(gs // 2):
            pi = b0 // 2 + half
            for par in range(2):
                m = half * 2 + par
                rows = slice(par * 64, par * 64 + 64)
                nc.vector.tensor_copy(
                    out=w_sh[rows, pi, m : m + 1], in_=ps_t[rows, m : m + 1]
                )

    def emit_matvec_half(b, half):
        """Weighted-sum matmuls for batch b, split in two so independent PE
        work can be interleaved across the softmax chain.  half 0: private
        tiles 0-1; half 1: private tiles 2-3 + the pair-shared tail."""
        gi, b0, gs, bl = _B2G[b]
        if gi not in mv_state:
            mv_state[gi] = [
                psB.tile([GSMAX, 512], F32, tag="mv", name=f"ps_mv_{gi}_{n}")
                for n in range(NN)
            ]
        ps_mv = mv_state[gi]
        ti = half
        s0, fu = FTS_PRIV[ti]
        ft = ft_tiles.pop((b, ti))
        for u in range(fu):
            t = s0 + u
            for n in range(NN):
                nc.tensor.matmul(
                    ps_mv[n][:gs],
                    lhsT=w_mask[:, t, b, b0 : b0 + gs],
                    rhs=ft[:, u, n * 512 : (n + 1) * 512],
                    start=(bl == 0 and t == 0),
                    stop=False,
                )
        if half == 1 and b % 2 == 1:
            pi = b // 2
            fsh = fsh_tiles.pop(pi)
            for n in range(NN):
                nc.tensor.matmul(
                    ps_mv[n][:gs],
                    lhsT=w_sh[:, pi, 0:gs],
                    rhs=fsh[:, n * 512 : (n + 1) * 512],
                    start=False,
                    stop=(bl == gs - 1),
                )

    def finish_matvec(gi):
        """Scale by 1/sum during PSUM evacuation and store the group.
        Split across ScalarE and DVE so the PSUM banks free up fast (the
        next group's first matmul waits on them)."""
        b0, gs = GROUPS[gi]
        ps_mv = mv_state[gi]
        out_sb = mlp.tile([GSMAX, RNN], F32, tag="out_sb", bufs=2, name=f"out_sb{gi}")
        for n in range(NN):
            dst = out_sb[:gs, n * 512 : (n + 1) * 512]
            if n % 2 == 0:
                nc.vector.tensor_scalar_mul(
                    out=dst, in0=ps_mv[n][:gs], scalar1=rs_g[gi][:gs]
                )
            else:
                nc.scalar.activation(
                    out=dst, in_=ps_mv[n][:gs], func=COPY, scale=rs_g[gi][:gs]
                )
        nc.sync.dma_start(out=io["out"][b0 : b0 + gs, :], in_=out_sb[:gs])

    # ---- flat batch-level pipeline ----
    # The weighted sum trails the scores by one group; `owed` holds batches
    # whose matvec is still due.  Each iteration drains its share of the
    # previous group's matvecs, split around the current scores batch so the
    # PE has independent work queued across every group's softmax chain.
    from collections import deque

    owed = deque()  # (batch, half) units of matvec work still due
    fin_mv = []  # matvec groups whose evacuation is deferred one iteration

    def emit_one_half(unit):
        mb, half = unit
        emit_matvec_half(mb, half)
        mgi, _, mgs, mbl = _B2G[mb]
        if half == 1 and mbl == mgs - 1:
            fin_mv.append(mgi)
    for b in range(PRE_PT):
        emit_pt_dma(b)
    emit_tanh(0)
    for b in range(PRE_FT):
        emit_ft_dma(b)
    for b in range(BPC):
        gi, b0, gs, bl = _B2G[b]
        while fin_mv:
            finish_matvec(fin_mv.pop())
        if b + PRE_PT < BPC:
            emit_pt_dma(b + PRE_PT)
        if b + PRE_FT < BPC:
            emit_ft_dma(b + PRE_FT)
        boundary = bl == gs - 1
        if not boundary and b + 1 < BPC:
            # at group boundaries the tanh is emitted after the softmax
            # chain so the in-order ScalarE runs EXP first
            emit_tanh(b + 1)
        # drain the owed matvec halves evenly across the group's iterations
        # so the PE has steady independent work, including across the
        # softmax chain at the boundary
        rem = gs - bl
        k = -(-len(owed) // rem) if owed else 0
        todo = [owed.popleft() for _ in range(k)]
        if todo:
            emit_one_half(todo[0])
        emit_scores_batch(b)
        for j, unit in enumerate(todo):
            if j > 0:
                emit_one_half(unit)
        if boundary:
            finish_scores(gi)
            for mb in range(b0, b0 + gs):
                owed.append((mb, 0))
                owed.append((mb, 1))
            if b + 1 < BPC:
                emit_tanh(b + 1)
    while fin_mv:
        finish_matvec(fin_mv.pop())
    for unit in owed:
        emit_one_half(unit)
    finish_matvec(NGRP - 1)


def _build():
    from contextlib import ExitStack

    nc = bacc.Bacc("TRN2", target_bir_lowering=False, debug=False, num_devices=N_CORES)
    io = {
        # pre-tiled: [partition][k][b]
        "hT": nc.dram_tensor(
            "hT", [128, KT // 2, 2, BPC], FP8E4, kind="ExternalInput"
        ).ap(),
        # pre-tiled: [batch][partition][ht][s]
        "pT": nc.dram_tensor("pT", [BPC, 128, NHT, SE], FP8, kind="ExternalInput").ap(),
        # pre-tiled private pairs: [batch][pair][partition][u][d]
        "f2": nc.dram_tensor(
            "f2", [BPC, 2, 128, 2, RNN], FP8, kind="ExternalInput"
        ).ap(),
        # pair-shared tail tiles: [pair][partition][d]
        "fsh": nc.dram_tensor("fsh", [NPAIR, 128, RNN], FP8, kind="ExternalInput").ap(),
        "madd": nc.dram_tensor(
            "madd", [GSMAX, NGRP, SE], BF16, kind="ExternalInput"
        ).ap(),
        # pre-tiled: [chunk][partition][u][o]
        "wct": nc.dram_tensor(
            "wct", [NWC, 128, 2, 2, HID], FP8E4, kind="ExternalInput"
        ).ap(),
        "bc": nc.dram_tensor("bc", [1, HID], BF16, kind="ExternalInput").ap(),
        "warep": nc.dram_tensor(
            "warep", [128, NHT * BPC * BPC], BF16, kind="ExternalInput"
        ).ap(),
        "out": nc.dram_tensor("out", [BPC, RNN], F32, kind="ExternalOutput").ap(),
    }
    with tile.TileContext(nc) as tc:
        with ExitStack() as ctx:
            _build_body(ctx, tc, io)
    nc.compile()
    return nc


@functools.lru_cache(maxsize=1)
def _get_nc():
    return _build()


def _prep_in_maps(h, att_feats, p_att_feats, mask, W1, b1, W2, b2, W3, b3, W4, b4, Wa, ba):
    f32 = np.float32
    bf16 = ml_dtypes.bfloat16
    asc = np.ascontiguousarray

    W1, W2, W3, W4 = (np.asarray(w, dtype=f32) for w in (W1, W2, W3, W4))
    b1, b2, b3, b4 = (np.asarray(b, dtype=f32) for b in (b1, b2, b3, b4))
    # Constant-fold the 4 linear layers (no nonlinearity between them):
    # att_h = h @ Wc.T + bc.  Scaled by WC_SCALE to clear fp8 subnormals;
    # the kernel divides by WC_SCALE during PSUM evacuation.
    Wc = W4 @ W3 @ W2 @ W1  # [HID, RNN]
    bc = ((b1 @ W2.T + b2) @ W3.T + b3) @ W4.T + b4  # [HID]
    # pre-tiled [NWC][128][kpair][u][HID] with k-pairs interleaved for the
    # DoubleRow matmul; e4m3 (values are within the TRN/OCP-common range)
    wct = asc(
        (Wc.T * WC_SCALE).reshape(NWC, 2, 2, 128, HID).transpose(0, 3, 1, 2, 4)
    ).astype(NP_FP8E4)
    bcr = (bc * WC_SCALE).astype(bf16).reshape(1, -1)

    wa = np.asarray(Wa, dtype=f32).reshape(-1)  # [HID]
    warep = np.zeros((128, NHT, BPC, BPC), dtype=f32)
    for ht in range(NHT):
        for b in range(BPC):
            warep[:, ht, b, b] = wa[ht * 128 : (ht + 1) * 128]
    warep = warep.reshape(128, NHT * BPC * BPC).astype(bf16)

    h = np.asarray(h, dtype=f32)
    p = np.asarray(p_att_feats, dtype=f32)
    f = np.asarray(att_feats, dtype=f32)
    m = np.asarray(mask)

    # Mask compaction onto the virtual SE=640 score axis.  Even batches put
    # their kept rows at columns [0, count); odd batches at [0, min(count,
    # 512)) plus [576, 576 + max(0, count-512)).  Everything else is pad.
    keep = m == 0
    counts = keep.sum(axis=1)  # [B]
    assert int(counts.max()) <= SB, (
        f"per-batch unmasked count {int(counts.max())} exceeds SB={SB}"
    )
    idx = np.zeros((B, SE), dtype=np.int64)
    madd_all = np.full((B, SE), MASK_NEG, dtype=f32)
    for gb in range(B):
        real = np.nonzero(keep[gb])[0]
        c = len(real)
        if gb % 2 == 0:
            idx[gb, :c] = real
            madd_all[gb, :c] = 0.0
        else:
            c0 = min(c, 512)
            idx[gb, :c0] = real[:c0]
            madd_all[gb, :c0] = 0.0
            if c > 512:
                idx[gb, SB : SB + c - 512] = real[512:]
                madd_all[gb, SB : SB + c - 512] = 0.0
    madd_all = madd_all.astype(bf16)

    in_maps = []
    for cix in range(N_CORES):
        sl = slice(cix * BPC, (cix + 1) * BPC)
        bidx = np.arange(cix * BPC, (cix + 1) * BPC)[:, None]
        idx_c = idx[sl]
        # private tiles: columns 0-511 -> [BPC, 2, 128, 2, RNN]
        f_priv = f[bidx, idx_c[:, :512]].astype(NP_FP8)  # [BPC, 512, RNN]
        f2 = asc(f_priv.reshape(BPC, 2, 2, 128, RNN).transpose(0, 1, 3, 2, 4))
        # shared tail tiles: rows 0-63 = even batch cols 512-575, rows
        # 64-127 = odd batch cols 576-639
        fsh = np.empty((NPAIR, 128, RNN), dtype=NP_FP8)
        for pi in range(NPAIR):
            be = cix * BPC + 2 * pi
            fsh[pi, 0:64] = f[be, idx[be, 512:SB]].astype(NP_FP8)
            fsh[pi, 64:128] = f[be + 1, idx[be + 1, SB:SE]].astype(NP_FP8)
        p_c = p[bidx, idx_c]  # [BPC, SE, HID]
        pT_c = asc(
            p_c.transpose(0, 2, 1).reshape(BPC, NHT, 128, SE).transpose(0, 2, 1, 3)
        ).astype(NP_FP8)
        # madd regrouped to [GSMAX, NGRP, SE] with group bl on partitions
        madd = madd_all[sl]
        madd_g = np.zeros((GSMAX, NGRP, SE), dtype=bf16)
        for gi, (b0, gs) in enumerate(GROUPS):
            madd_g[:gs, gi] = madd[b0 : b0 + gs]
        # pre-tiled hT: [p][kpair][u][b] = h[b, (2*kpair+u)*128+p]
        hT = asc(
            h[sl].T.reshape(KT // 2, 2, 128, BPC).transpose(2, 0, 1, 3)
        ).astype(NP_FP8E4)
        in_maps.append(
            {
                "hT": hT,
                "pT": pT_c,
                "f2": f2,
                "fsh": fsh,
                "madd": madd_g,
                "wct": wct,
                "bc": bcr,
                "warep": warep,
            }
        )
    return in_maps, SE


def _run(in_maps, se=SE, trace=False):
    nc = _get_nc()
    res = run_bass_kernel_spmd(nc, in_maps, core_ids=list(range(N_CORES)), trace=trace)
    out = np.concatenate([res.results[c]["out"] for c in range(N_CORES)], axis=0)
    return out, res


def kernel(h, att_feats, p_att_feats, mask, W1, b1, W2, b2, W3, b3, W4, b4, Wa, ba):
    in_maps, se = _prep_in_maps(
        h, att_feats, p_att_feats, mask, W1, b1, W2, b2, W3, b3, W4, b4, Wa, ba
    )
    out, _ = _run(in_maps, se)
    return out


# revision 28
# speedup vs baseline: 1.0093x; 1.0093x over previous
"""Trainium2 Bass kernel for the Attention3 module (B=128, S=1024, RNN=2048, HID=512).

Strategy: data-parallel over batch B across 8 NeuronCores (16 batches/core),
plus structural optimizations:

  * Mask compaction (sparse attention): positions with mask==1 get softmax
    weight exactly 0, so their att_feats / p_att_feats rows are never read.
    The host gathers the unmasked rows per batch into a virtual score axis of
    SE=640 columns.  Each batch is padded to 576 positions; the tail 64 of a
    batch PAIR share one feature tile: the even batch's tail sits at score
    columns 512-575 (tile-4 partitions 0-63), the odd batch's at columns
    576-639 (partitions 64-127), so one [128, RNN] att_feats tile and one
    matmul serve both batches' tails.  Pad positions get an additive -1e9
    score so exp() zeroes them.
  * fp8 (e3m4) storage for att_feats, p_att_feats^T and the folded MLP
    weight.  The PE accepts mixed bf16-stationary x fp8-moving matmuls, and
    ScalarE auto-upconverts the fp8 tanh input.  Rel err ~1.4e-2 (gate 2e-2).
  * The 4 MLP layers have no nonlinearity between them, so the host folds
    W4@W3@W2@W1 into one [512, 2048] matrix, scaled by 256 to clear the fp8
    subnormal range (un-scaled during PSUM evacuation).
  * Softmax skips the max-subtraction: real scores are O(1) (sum of 512
    tanh-bounded terms times 0.02-scale weights), so exp() cannot overflow,
    and the -1e9 padding still underflows to exactly 0.  This shortens the
    serial softmax chain at each group boundary.

All large streams are stored PRE-TILED in HBM (partition-major within each
DMA tile) so every DMA descriptor is a 0.5-4 KiB contiguous per-partition
line.

Per-core device pipeline: batches run in groups of sizes [2,2,4,4,2,2]
(small groups at the ends shorten the fill and drain of the two-stage
scores->weighted-sum pipeline; big groups in the middle keep PE streaks
long).  Scores of group g overlap the weighted sum of group g-1; each
weighted-sum batch is emitted in two halves around the next scores batch so
the PE always has independent work queued across a group's softmax chain.
  1. MLP: att_h = h@Wc.T + bc (PE, f32 acc), bias folded in as a K=1
     ones-outer-product matmul appended to the same PSUM group.
  2. scores: tanh(p_att^T + att_h) with HID on partitions; ScalarE reads the
     fp8 p tile and writes a bf16 tile with att_h as per-partition bias; the
     Wa contraction is a PE matmul whose stationary column holds Wa masked to
     batch b, so each batch accumulates into its own PSUM row.
  3. softmax: exp (unnormalized, no max-sub) straight out of PSUM; the exp
     output is PE-transposed onto the block-diagonal of the masked weight
     tensor (full tiles) and onto per-pair split columns (shared tail tile);
     1/sum is folded into the final PSUM evacuation.
  4. weighted sum: stream compacted fp8 att_feats tiles and matmul against
     the bf16 block-diagonal weights; each batch lands in its own PSUM row.

DMA queues: wct/bc then pt tiles + consts ride the ACT HWDGE ring
(nc.scalar), so the MLP inputs land inside ~10us; f tiles round-robin
sync/gpsimd.
"""

import functools

import ml_dtypes
import numpy as np

import concourse.bacc as bacc
import concourse.bass as bass
import concourse.tile as tile
from concourse import mybir
from concourse.bass_utils import run_bass_kernel_spmd
from concourse.masks import make_identity

N_CORES = 8
B, S, RNN, HID = 128, 1024, 2048, 512
BPC = B // N_CORES  # batches per core
NPAIR = BPC // 2  # tail-sharing batch pairs
GROUPS = ((0, 2), (2, 2), (4, 4), (8, 4), (12, 2), (14, 2))  # (start, size)
NGRP = len(GROUPS)
GSMAX = 4
F32 = mybir.dt.float32
BF16 = mybir.dt.bfloat16
FP8 = mybir.dt.float8e3
FP8E4 = mybir.dt.float8e4
NP_FP8 = ml_dtypes.float8_e3m4
NP_FP8E4 = ml_dtypes.float8_e4m3
MASK_NEG = -1.0e9
WC_SCALE = 256.0
SE = 640  # virtual score-axis length
SB = 576  # per-batch padded position count
NFULL = 4  # full 128-position feature tiles per batch
AX_X = mybir.AxisListType.X
TANH = mybir.ActivationFunctionType.Tanh
EXP = mybir.ActivationFunctionType.Exp
COPY = mybir.ActivationFunctionType.Copy

NHT = HID // 128  # 4 h-tiles
NN = RNN // 512  # 4 output chunks
KT = RNN // 128  # 16 k-tiles in the MLP contraction
NWC = 4  # wct DMA chunks (4 k-tiles each)
FTS_PRIV = ((0, 2), (2, 2))  # private f tiles per batch: (start, count)

PRE_PT = 3  # p tiles issued this many batches ahead
PRE_FT = 3  # f tiles issued this many batches ahead

# batch -> (group_index, start, size, offset_in_group)
_B2G = {}
for _gi, (_b0, _gs) in enumerate(GROUPS):
    for _bl in range(_gs):
        _B2G[_b0 + _bl] = (_gi, _b0, _gs, _bl)


def _build_body(ctx, tc, io):
    nc = tc.nc

    consts = ctx.enter_context(tc.tile_pool(name="consts", bufs=1))
    wpool = ctx.enter_context(tc.tile_pool(name="wpool", bufs=4))
    mlp = ctx.enter_context(tc.tile_pool(name="mlp", bufs=1))
    ppool = ctx.enter_context(tc.tile_pool(name="ppool", bufs=5))
    # ft DMAs ride only sync/gpsimd (never a compute-critical queue: a
    # ring-full ft dma_start waiting on its matvec consumer must not block
    # ops that the matvec itself depends on).
    fpool = ctx.enter_context(tc.tile_pool(name="fpool", bufs=16))
    psA = ctx.enter_context(tc.tile_pool(name="psA", bufs=4, space="PSUM"))
    psB = ctx.enter_context(tc.tile_pool(name="psB", bufs=4, space="PSUM"))

    # ---- the MLP-critical inputs go out first, on the fast ACT ring ----
    wct_sb = []
    for wc in range(NWC):
        wt = wpool.tile([128, 2, 2, HID], FP8E4, tag="wt", name=f"wt{wc}")
        nc.scalar.dma_start(out=wt, in_=io["wct"][wc])
        wct_sb.append(wt)
    bc_sb = consts.tile([1, HID], BF16)
    nc.scalar.dma_start(out=bc_sb, in_=io["bc"])
    hT_sb = consts.tile([128, KT // 2, 2, BPC], FP8E4)
    nc.sync.dma_start(out=hT_sb, in_=io["hT"])

    # ---- constants / small inputs ----
    ident = consts.tile([128, 128], F32)
    make_identity(nc, ident)
    ones_f = consts.tile([1, BPC], F32)
    nc.vector.memset(ones_f, 1.0)
    ones1 = consts.tile([1, BPC], BF16)
    nc.vector.tensor_copy(out=ones1, in_=ones_f)

    # ---- phase 1: folded MLP, one layer (DoubleRow fp8, f32 accumulate;
    # this runs cold at mid p-state so halving the rows matters) ----
    ps_ah = psA.tile([BPC, HID], F32, tag="ps_small", name="ps_ah")
    for wc in range(NWC):
        for kpi in range(2):
            kp = wc * 2 + kpi
            nc.tensor.matmul(
                ps_ah,
                lhsT=hT_sb[:, kp, :, :],
                rhs=wct_sb[wc][:, kpi, :, :],
                start=(kp == 0),
                stop=False,
                perf_mode=mybir.MatmulPerfMode.DoubleRow,
            )
    # bias last so bc's DMA stays off the critical path; wct is scaled by
    # WC_SCALE (fp8 subnormal avoidance), bc rides in pre-scaled, and the
    # evacuation divides both out.
    nc.tensor.matmul(ps_ah, lhsT=ones1, rhs=bc_sb, start=False, stop=True)
    ah = mlp.tile([BPC, HID], F32, tag="ah")
    nc.vector.tensor_scalar_mul(out=ah, in0=ps_ah, scalar1=1.0 / WC_SCALE)
    ahT = mlp.tile([128, NHT, BPC], F32, tag="ahT")
    for j in range(NHT):
        ps = psA.tile([128, BPC], F32, tag="ps_small", name=f"ps_tr_ah{j}")
        nc.tensor.transpose(ps, ah[:, j * 128 : (j + 1) * 128], ident[:BPC, :BPC])
        nc.vector.tensor_copy(out=ahT[:, j, :], in_=ps)

    # Block-diagonal masked softmax weights, zeroed early: w_mask[:, t, b, m]
    # = exp_w[s, b] if m == b else 0 (full tiles t<4); w_sh[:, pi, m] holds
    # the pair-shared tail tile (rows 0-63 even batch, 64-127 odd batch).
    w_mask = mlp.tile([128, NFULL, BPC, BPC], BF16, tag="w_mask")
    nc.vector.memset(w_mask, 0.0)
    w_sh = mlp.tile([128, NPAIR, GSMAX], BF16, tag="w_sh")
    nc.vector.memset(w_sh, 0.0)

    wa_sb = consts.tile([128, NHT * BPC * BPC], BF16)
    nc.sync.dma_start(out=wa_sb, in_=io["warep"])
    wa_m = wa_sb.rearrange("p (t b m) -> p t b m", t=NHT, b=BPC)

    madd_sb = consts.tile([GSMAX, NGRP, SE], BF16)
    nc.sync.dma_start(out=madd_sb, in_=io["madd"])

    # Per-group state for the batch-interleaved pipeline below.
    sc_state = {}
    mv_state = {}
    rs_g = {}
    pt_tiles = {}
    ptt_tiles = {}
    ft_tiles = {}
    fsh_tiles = {}
    ft_ctr = [0]
    f_engs = [nc.sync, nc.gpsimd]

    def fq():
        eng = f_engs[ft_ctr[0] % 2]
        ft_ctr[0] += 1
        return eng

    def emit_pt_dma(b):
        # pt rides the ACT ring: it must never queue behind the ft stream,
        # whose ring-full waits would delay the tanh chain
        pt = ppool.tile([128, NHT, SE], FP8, tag="pt", name=f"pt_{b}")
        nc.scalar.dma_start(out=pt, in_=io["pT"][b])
        pt_tiles[b] = pt

    def emit_ft_dma(b):
        """Private f tiles for batch b + the shared tail tile on odd b.

        The first five batches ride the ACT ring BEHIND the wct chunks:
        their transfers then cannot start until the MLP weights have landed,
        so the fill-critical wct stream gets the DMA engines to itself.
        This is safe (no ring-full stall on the compute queue) because those
        early tiles are all fresh fpool slots."""
        early = b <= 4

        def eng():
            return nc.scalar if early else fq()

        for ti, (s0, fu) in enumerate(FTS_PRIV):
            ft = fpool.tile([128, 2, RNN], FP8, tag="ft", name=f"ft_{b}_{ti}")
            eng().dma_start(out=ft, in_=io["f2"][b, ti])
            ft_tiles[(b, ti)] = ft
        if b % 2 == 1:
            pi = b // 2
            fsh = fpool.tile([128, RNN], FP8, tag="fsh", bufs=4, name=f"fsh_{pi}")
            eng().dma_start(out=fsh, in_=io["fsh"][pi])
            fsh_tiles[pi] = fsh

    def emit_tanh(b):
        # Even batches only occupy score columns [0, 576); odd ones also use
        # the [576, 640) tail (their interior [512, 576) stays pad-garbage,
        # zeroed later by exp(-1e9)).
        pt = pt_tiles.pop(b)
        ptt = ppool.tile([128, NHT, SE], BF16, tag="ptt", bufs=4, name=f"ptt_{b}")
        hi = SB if b % 2 == 0 else SE
        for ht in range(NHT):
            nc.scalar.activation(
                out=ptt[:, ht, 0:hi],
                in_=pt[:, ht, 0:hi],
                func=TANH,
                bias=ahT[:, ht, b : b + 1],
                scale=1.0,
            )
        ptt_tiles[b] = ptt

    def emit_scores_batch(b):
        gi, b0, gs, bl = _B2G[b]
        if gi not in sc_state:
            sc_state[gi] = {
                "m": psA.tile([GSMAX, 512], F32, tag="ps_small", name=f"ps_sc_{gi}"),
                "e": psA.tile([GSMAX, 64], F32, tag="ps_small", name=f"ps_se_{gi}"),
                "o": psA.tile([GSMAX, 64], F32, tag="ps_small", name=f"ps_so_{gi}"),
            }
        ps = sc_state[gi]
        ptt = ptt_tiles.pop(b)
        even = b % 2 == 0
        off, pst = (512, ps["e"]) if even else (SB, ps["o"])
        for ht in range(NHT):
            nc.tensor.matmul(
                ps["m"][:gs],
                lhsT=wa_m[:, ht, b, b0 : b0 + gs],
                rhs=ptt[:, ht, 0:512],
                start=(bl == 0 and ht == 0),
                stop=(bl == gs - 1 and ht == NHT - 1),
            )
            nc.tensor.matmul(
                pst[:gs],
                lhsT=wa_m[:, ht, b, b0 : b0 + gs],
                rhs=ptt[:, ht, off : off + 64],
                start=(bl == (0 if even else 1) and ht == 0),
                stop=(bl == (gs - 2 if even else gs - 1) and ht == NHT - 1),
            )

    def finish_scores(gi):
        """Evacuate score PSUM (+mask), exp without max-sub, write the
        masked-weight diagonals."""
        b0, gs = GROUPS[gi]
        ps = sc_state[gi]
        scores = mlp.tile([GSMAX, SE], F32, tag="scores", bufs=2, name=f"scores{gi}")
        for off, w, key in ((0, 512, "m"), (512, 64, "e"), (SB, 64, "o")):
            nc.vector.tensor_add(
                out=scores[:gs, off : off + w],
                in0=ps[key][:gs],
                in1=madd_sb[:gs, gi, off : off + w],
            )
        ssum = mlp.tile([GSMAX, 1], F32, tag="ssum", bufs=2, name=f"ssum{gi}")
        nc.scalar.activation(
            out=scores[:gs], in_=scores[:gs], func=EXP, scale=1.0, accum_out=ssum[:gs]
        )
        rs = mlp.tile([GSMAX, 1], F32, tag="rs", bufs=2, name=f"rs{gi}")
        nc.vector.reciprocal(out=rs[:gs], in_=ssum[:gs])
        rs_g[gi] = rs
        for t in range(NFULL):
            ps_t = psA.tile([128, GSMAX], F32, tag="ps_small", name=f"ps_tr{gi}_{t}")
            nc.tensor.transpose(
                ps_t[:, :gs], scores[:gs, t * 128 : (t + 1) * 128], ident[:gs, :gs]
            )
            sl = w_mask[:, t, :, :]
            diag_ap = bass.AP(
                tensor=sl.tensor,
                offset=sl.offset + b0 * (BPC + 1),
                ap=[sl.ap[0], [BPC + 1, gs]],
            )
            nc.vector.tensor_copy(out=diag_ap, in_=ps_t[:, :gs])
        # shared tail tile: columns 512-639 transpose to partitions 0-127;
        # rows 0-63 belong to the even batch of each pair, 64-127 to the odd.
        ps_t = psA.tile([128, GSMAX], F32, tag="ps_small", name=f"ps_trs{gi}")
        nc.tensor.transpose(ps_t[:, :gs], scores[:gs, 512:SE], ident[:gs, :gs])
        for half in range(gs // 2):
            pi = b0 // 2 + half
            for par in range(2):
                m = half * 2 + par
                rows = slice(par * 64, par * 64 + 64)
                nc.vector.tensor_copy(
                    out=w_sh[rows, pi, m : m + 1], in_=ps_t[rows, m : m + 1]
                )

    def emit_matvec_half(b, half):
        """Weighted-sum matmuls for batch b, split in two so independent PE
        work can be interleaved across the softmax chain.  half 0: private
        tiles 0-1; half 1: private tiles 2-3 + the pair-shared tail."""
        gi, b0, gs, bl = _B2G[b]
        if gi not in mv_state:
            mv_state[gi] = [
                psB.tile([GSMAX, 512], F32, tag="mv", name=f"ps_mv_{gi}_{n}")
                for n in range(NN)
            ]
        ps_mv = mv_state[gi]
        ti = half
        s0, fu = FTS_PRIV[ti]
        ft = ft_tiles.pop((b, ti))
        for u in range(fu):
            t = s0 + u
            for n in range(NN):
                nc.tensor.matmul(
                    ps_mv[n][:gs],
                    lhsT=w_mask[:, t, b, b0 : b0 + gs],
                    rhs=ft[:, u, n * 512 : (n + 1) * 512],
                    start=(bl == 0 and t == 0),
                    stop=False,
                )
        if half == 1 and b % 2 == 1:
            pi = b // 2
            fsh = fsh_tiles.pop(pi)
            for n in range(NN):
                nc.tensor.matmul(
                    ps_mv[n][:gs],
                    lhsT=w_sh[:, pi, 0:gs],
                    rhs=fsh[:, n * 512 : (n + 1) * 512],
                    start=False,
                    stop=(bl == gs - 1),
                )

    def finish_matvec(gi):
        """Scale by 1/sum during PSUM evacuation and store the group.
        Split across ScalarE and DVE so the PSUM banks free up fast (the
        next group's first matmul waits on them)."""
        b0, gs = GROUPS[gi]
        ps_mv = mv_state[gi]
        out_sb = mlp.tile([GSMAX, RNN], F32, tag="out_sb", bufs=2, name=f"out_sb{gi}")
        for n in range(NN):
            dst = out_sb[:gs, n * 512 : (n + 1) * 512]
            if n % 2 == 0:
                nc.vector.tensor_scalar_mul(
                    out=dst, in0=ps_mv[n][:gs], scalar1=rs_g[gi][:gs]
                )
            else:
                nc.scalar.activation(
                    out=dst, in_=ps_mv[n][:gs], func=COPY, scale=rs_g[gi][:gs]
                )
        nc.sync.dma_start(out=io["out"][b0 : b0 + gs, :], in_=out_sb[:gs])

    # ---- flat batch-level pipeline ----
    # The weighted sum trails the scores by one group; `owed` holds batches
    # whose matvec is still due.  Each iteration drains its share of the
    # previous group's matvecs, split around the current scores batch so the
    # PE has independent work queued across every group's softmax chain.
    from collections import deque

    owed = deque()  # (batch, half) units of matvec work still due
    fin_mv = []  # matvec groups whose evacuation is deferred one iteration

    def emit_one_half(unit):
        mb, half = unit
        emit_matvec_half(mb, half)
        mgi, _, mgs, mbl = _B2G[mb]
        if half == 1 and mbl == mgs - 1:
            fin_mv.append(mgi)
    for b in range(PRE_PT):
        emit_pt_dma(b)
    emit_tanh(0)
    for b in range(PRE_FT):
        emit_ft_dma(b)
    for b in range(BPC):
        gi, b0, gs, bl = _B2G[b]
        while fin_mv:
            finish_matvec(fin_mv.pop())
        if b + PRE_PT < BPC:
            emit_pt_dma(b + PRE_PT)
        if b + PRE_FT < BPC:
            emit_ft_dma(b + PRE_FT)
        boundary = bl == gs - 1
        if not boundary and b + 1 < BPC:
            # at group boundaries the tanh is emitted after the softmax
            # chain so the in-order ScalarE runs EXP first
            emit_tanh(b + 1)
        # drain the owed matvec halves evenly across the group's iterations
        # so the PE has steady independent work, including across the
        # softmax chain at the boundary
        rem = gs - bl
        k = -(-len(owed) // rem) if owed else 0
        todo = [owed.popleft() for _ in range(k)]
        if todo:
            emit_one_half(todo[0])
        emit_scores_batch(b)
        for j, unit in enumerate(todo):
            if j > 0:
                emit_one_half(unit)
        if boundary:
            finish_scores(gi)
            for mb in range(b0, b0 + gs):
                owed.append((mb, 0))
                owed.append((mb, 1))
            if b + 1 < BPC:
                emit_tanh(b + 1)
    while fin_mv:
        finish_matvec(fin_mv.pop())
    for unit in owed:
        emit_one_half(unit)
    finish_matvec(NGRP - 1)


def _build():
    from contextlib import ExitStack

    nc = bacc.Bacc("TRN2", target_bir_lowering=False, debug=False, num_devices=N_CORES)
    io = {
        # pre-tiled: [partition][k][b]
        "hT": nc.dram_tensor(
            "hT", [128, KT // 2, 2, BPC], FP8E4, kind="ExternalInput"
        ).ap(),
        # pre-tiled: [batch][partition][ht][s]
        "pT": nc.dram_tensor("pT", [BPC, 128, NHT, SE], FP8, kind="ExternalInput").ap(),
        # pre-tiled private pairs: [batch][pair][partition][u][d]
        "f2": nc.dram_tensor(
            "f2", [BPC, 2, 128, 2, RNN], FP8, kind="ExternalInput"
        ).ap(),
        # pair-shared tail tiles: [pair][partition][d]
        "fsh": nc.dram_tensor("fsh", [NPAIR, 128, RNN], FP8, kind="ExternalInput").ap(),
        "madd": nc.dram_tensor(
            "madd", [GSMAX, NGRP, SE], BF16, kind="ExternalInput"
        ).ap(),
        # pre-tiled: [chunk][partition][u][o]
        "wct": nc.dram_tensor(
            "wct", [NWC, 128, 2, 2, HID], FP8E4, kind="ExternalInput"
        ).ap(),
        "bc": nc.dram_tensor("bc", [1, HID], BF16, kind="ExternalInput").ap(),
        "warep": nc.dram_tensor(
            "warep", [128, NHT * BPC * BPC], BF16, kind="ExternalInput"
        ).ap(),
        "out": nc.dram_tensor("out", [BPC, RNN], F32, kind="ExternalOutput").ap(),
    }
    with tile.TileContext(nc) as tc:
        with ExitStack() as ctx:
            _build_body(ctx, tc, io)
    nc.compile()
    return nc


@functools.lru_cache(maxsize=1)
def _get_nc():
    return _build()


def _prep_in_maps(h, att_feats, p_att_feats, mask, W1, b1, W2, b2, W3, b3, W4, b4, Wa, ba):
    f32 = np.float32
    bf16 = ml_dtypes.bfloat16
    asc = np.ascontiguousarray

    W1, W2, W3, W4 = (np.asarray(w, dtype=f32) for w in (W1, W2, W3, W4))
    b1, b2, b3, b4 = (np.asarray(b, dtype=f32) for b in (b1, b2, b3, b4))
    # Constant-fold the 4 linear layers (no nonlinearity between them):
    # att_h = h @ Wc.T + bc.  Scaled by WC_SCALE to clear fp8 subnormals;
    # the kernel divides by WC_SCALE during PSUM evacuation.
    Wc = W4 @ W3 @ W2 @ W1  # [HID, RNN]
    bc = ((b1 @ W2.T + b2) @ W3.T + b3) @ W4.T + b4  # [HID]
    # pre-tiled [NWC][128][kpair][u][HID] with k-pairs interleaved for the
    # DoubleRow matmul; e4m3 (values are within the TRN/OCP-common range)
    wct = asc(
        (Wc.T * WC_SCALE).reshape(NWC, 2, 2, 128, HID).transpose(0, 3, 1, 2, 4)
    ).astype(NP_FP8E4)
    bcr = (bc * WC_SCALE).astype(bf16).reshape(1, -1)

    wa = np.asarray(Wa, dtype=f32).reshape(-1)  # [HID]
    warep = np.zeros((128, NHT, BPC, BPC), dtype=f32)
    for ht in range(NHT):
        for b in range(BPC):
            warep[:, ht, b, b] = wa[ht * 128 : (ht + 1) * 128]
    warep = warep.reshape(128, NHT * BPC * BPC).astype(bf16)

    h = np.asarray(h, dtype=f32)
    p = np.asarray(p_att_feats, dtype=f32)
    f = np.asarray(att_feats, dtype=f32)
    m = np.asarray(mask)

    # Mask compaction onto the virtual SE=640 score axis.  Even batches put
    # their kept rows at columns [0, count); odd batches at [0, min(count,
    # 512)) plus [576, 576 + max(0, count-512)).  Everything else is pad.
    keep = m == 0
    counts = keep.sum(axis=1)  # [B]
    assert int(counts.max()) <= SB, (
        f"per-batch unmasked count {int(counts.max())} exceeds SB={SB}"
    )
    idx = np.zeros((B, SE), dtype=np.int64)
    madd_all = np.full((B, SE), MASK_NEG, dtype=f32)
    for gb in range(B):
        real = np.nonzero(keep[gb])[0]
        c = len(real)
        if gb % 2 == 0:
            idx[gb, :c] = real
            madd_all[gb, :c] = 0.0
        else:
            c0 = min(c, 512)
            idx[gb, :c0] = real[:c0]
            madd_all[gb, :c0] = 0.0
            if c > 512:
                idx[gb, SB : SB + c - 512] = real[512:]
                madd_all[gb, SB : SB + c - 512] = 0.0
    madd_all = madd_all.astype(bf16)

    in_maps = []
    for cix in range(N_CORES):
        sl = slice(cix * BPC, (cix + 1) * BPC)
        bidx = np.arange(cix * BPC, (cix + 1) * BPC)[:, None]
        idx_c = idx[sl]
        # private tiles: columns 0-511 -> [BPC, 2, 128, 2, RNN]
        f_priv = f[bidx, idx_c[:, :512]].astype(NP_FP8)  # [BPC, 512, RNN]
        f2 = asc(f_priv.reshape(BPC, 2, 2, 128, RNN).transpose(0, 1, 3, 2, 4))
        # shared tail tiles: rows 0-63 = even batch cols 512-575, rows
        # 64-127 = odd batch cols 576-639
        fsh = np.empty((NPAIR, 128, RNN), dtype=NP_FP8)
        for pi in range(NPAIR):
            be = cix * BPC + 2 * pi
            fsh[pi, 0:64] = f[be, idx[be, 512:SB]].astype(NP_FP8)
            fsh[pi, 64:128] = f[be + 1, idx[be + 1, SB:SE]].astype(NP_FP8)
        p_c = p[bidx, idx_c]  # [BPC, SE, HID]
        pT_c = asc(
            p_c.transpose(0, 2, 1).reshape(BPC, NHT, 128, SE).transpose(0, 2, 1, 3)
        ).astype(NP_FP8)
        # madd regrouped to [GSMAX, NGRP, SE] with group bl on partitions
        madd = madd_all[sl]
        madd_g = np.zeros((GSMAX, NGRP, SE), dtype=bf16)
        for gi, (b0, gs) in enumerate(GROUPS):
            madd_g[:gs, gi] = madd[b0 : b0 + gs]
        # pre-tiled hT: [p][kpair][u][b] = h[b, (2*kpair+u)*128+p]
        hT = asc(
            h[sl].T.reshape(KT // 2, 2, 128, BPC).transpose(2, 0, 1, 3)
        ).astype(NP_FP8E4)
        in_maps.append(
            {
                "hT": hT,
                "pT": pT_c,
                "f2": f2,
                "fsh": fsh,
                "madd": madd_g,
                "wct": wct,
                "bc": bcr,
                "warep": warep,
            }
        )
    return in_maps, SE


def _run(in_maps, se=SE, trace=False):
    nc = _get_nc()
    res = run_bass_kernel_spmd(nc, in_maps, core_ids=list(range(N_CORES)), trace=trace)
    out = np.concatenate([res.results[c]["out"] for c in range(N_CORES)], axis=0)
    return out, res


def kernel(h, att_feats, p_att_feats, mask, W1, b1, W2, b2, W3, b3, W4, b4, Wa, ba):
    in_maps, se = _prep_in_maps(
        h, att_feats, p_att_feats, mask, W1, b1, W2, b2, W3, b3, W4, b4, Wa, ba
    )
    out, _ = _run(in_maps, se)
    return out
